# revision 15
# baseline (speedup 1.0000x reference)
"""Trainium2 Bass kernel for an AttentionBlock (GroupNorm + single-head
self-attention + projection + residual), data-parallel over batch on 8
NeuronCores.

Reference computation (per batch element, S = H*W = 4096, C = 256):
    xn   = GroupNorm(x, groups=8, eps=1e-3) * gamma + beta
    q    = xn @ Wq + bq ; k = xn @ Wk + bk ; v = xn @ Wv + bv
    attn = softmax((q @ k^T) / sqrt(C))
    out  = attn @ v
    y    = xn + (out @ Wp + bp)

Layout strategy (per core, B/8 batch elements, software-pipelined):
  - x is loaded in natural [s, c] tiles and PE-transposed to x_T [c, s];
    the evacuation copy folds in the per-channel running sum; two large
    ACT Square passes over x_T produce the per-channel sum of squares.
  - GroupNorm group stats come from two tiny cross-partition matmuls
    with group indicator matrices; normalization is applied per-channel
    (partition scalars) on x_T, producing xn_b (bf16, residual path)
    and xn8 (fp8e4, QKV path).
  - All attention-sized matmuls (QKV, scores, attn@v) run as fp8e4
    DoubleRow matmuls: the two 128-deep contraction chunks are folded
    into a single PE pass (2 weights/cell), near-doubling PE throughput.
    q8/k8 in [c, s] layout, v8 in [t, c] layout, exp8 in [t, s] layout.
  - softmax skips max-subtraction but shifts exp by -3*ln2 so the
    largest exp value stays below fp8e4's 240 limit (the shift cancels
    between numerator and denominator). exp is written directly in fp8.
  - The denominator is ONE DoubleRow ones-matmul per key-tile pair
    accumulated into a persistent [1, SB] PSUM (no DVE tree), then the
    existing transpose-to-partition + reciprocal path.
  - Projection and residual stay bf16: oU evac bf16, Wp bf16, residual
    rides the projection PSUM phase as a PE transpose of xn_b.
  - Per query block, the finish chain (denominator transpose,
    projection, residual, store) is emitted after the NEXT block's
    key-loop; the next element's head is emitted during the current
    element's late attention blocks.
"""

import math
import os
import sys

for _p in ("/opt/trn_rl_repo", "/root/.axon_site/_ro/trn_rl_repo"):
    if os.path.isdir(_p) and _p not in sys.path:
        sys.path.append(_p)

import numpy as np
import ml_dtypes

import concourse.bass as bass
import concourse.mybir as mybir
import concourse.tile as tile
from concourse import bacc

F32 = mybir.dt.float32
BF16 = mybir.dt.bfloat16
FP8 = mybir.dt.float8e4
AF = mybir.ActivationFunctionType
AX = mybir.AxisListType
ALU = mybir.AluOpType
DR = mybir.MatmulPerfMode.DoubleRow

N_CORES = 8
EXP_BIAS = -3.0 * math.log(2.0)  # keep exp() under fp8e4 max (240)


def build_nc(B_loc=2, S=4096, C=256, G=8, EPS=1e-3, exp_bufs=3,
             use_bv=True, use_bp=True):
    """Build the single-core Bass program (SPMD: same program all cores)."""
    nc = bacc.Bacc(None, target_bir_lowering=False, debug=False)

    CK = C // 128          # channel chunks (2)
    NT = S // 128          # key tiles (32)
    NP = NT // 2           # key tile pairs (16)
    SB = 512               # query block size
    NSB = S // SB          # query blocks (8)
    att_scale = float(C) ** -0.5
    inv_n = 1.0 / float(S * (C // G))

    x_d = nc.dram_tensor("x", [B_loc, S, C], F32, kind="ExternalInput")
    y_d = nc.dram_tensor("y", [B_loc, S, C], F32, kind="ExternalOutput")
    wq_d = nc.dram_tensor("wq", [CK, 128, C], FP8, kind="ExternalInput")
    wk_d = nc.dram_tensor("wk", [CK, 128, C], FP8, kind="ExternalInput")
    wv_d = nc.dram_tensor("wv", [CK, 128, C], FP8, kind="ExternalInput")
    wp_d = nc.dram_tensor("wp", [CK, 128, C], BF16, kind="ExternalInput")
    bq_d = nc.dram_tensor("bq", [CK, 128, 1], F32, kind="ExternalInput")
    bk_d = nc.dram_tensor("bk", [CK, 128, 1], F32, kind="ExternalInput")
    bv_d = nc.dram_tensor("bv", [1, C], F32, kind="ExternalInput")
    bp_d = nc.dram_tensor("bp", [1, C], F32, kind="ExternalInput")
    gamma_d = nc.dram_tensor("gamma", [CK, 128, 1], F32, kind="ExternalInput")
    beta_d = nc.dram_tensor("beta", [CK, 128, 1], F32, kind="ExternalInput")
    ident_d = nc.dram_tensor("ident", [128, 128], F32, kind="ExternalInput")
    identb_d = nc.dram_tensor("identb", [128, 128], BF16, kind="ExternalInput")
    ones8_d = nc.dram_tensor("ones8", [128, 2, 16], FP8, kind="ExternalInput")
    expb_d = nc.dram_tensor("expb", [128, 1], F32, kind="ExternalInput")
    onesrow_d = nc.dram_tensor("onesrow", [1, 128], F32, kind="ExternalInput")
    one_d = nc.dram_tensor("one", [1, 1], F32, kind="ExternalInput")
    ind_d = nc.dram_tensor("ind", [CK, 128, G], F32, kind="ExternalInput")
    indt_d = nc.dram_tensor("indt", [CK, G, 128], F32, kind="ExternalInput")

    with tile.TileContext(nc) as tc:
        with (
            tc.tile_pool(name="sb", bufs=1) as sb,
            tc.tile_pool(name="pm", bufs=1, space="PSUM") as pm,
        ):
            # ---- load constants (resident for the whole kernel) ----
            def const_tile(shape, dtype, tag):
                return sb.tile(shape, dtype, tag=tag, bufs=1, name=tag)

            wq_sb = const_tile([128, CK, C], FP8, "wq")
            wk_sb = const_tile([128, CK, C], FP8, "wk")
            wv_sb = const_tile([128, CK, C], FP8, "wv")
            wp_sb = const_tile([128, CK, C], BF16, "wp")
            bq_sb = const_tile([128, CK], F32, "bq")
            bk_sb = const_tile([128, CK], F32, "bk")
            gamma_sb = const_tile([128, CK], F32, "gamma")
            beta_sb = const_tile([128, CK], F32, "beta")
            for b_sb, b_d in ((bq_sb, bq_d), (bk_sb, bk_d),
                              (gamma_sb, gamma_d), (beta_sb, beta_d)):
                for ck in range(CK):
                    nc.gpsimd.dma_start(b_sb[:, ck:ck + 1], b_d[ck])
            bv_sb = const_tile([1, C], F32, "bv")
            bp_sb = const_tile([1, C], F32, "bp")
            nc.gpsimd.dma_start(bv_sb[:], bv_d[:])
            nc.gpsimd.dma_start(bp_sb[:], bp_d[:])
            ident_sb = const_tile([128, 128], F32, "ident")
            nc.gpsimd.dma_start(ident_sb[:], ident_d[:])
            identb_sb = const_tile([128, 128], BF16, "identb")
            nc.gpsimd.dma_start(identb_sb[:], identb_d[:])
            ones8_sb = const_tile([128, 2, 16], FP8, "ones8")
            nc.gpsimd.dma_start(ones8_sb[:], ones8_d[:])
            expb_sb = const_tile([128, 1], F32, "expb")
            nc.gpsimd.dma_start(expb_sb[:], expb_d[:])
            onesrow_sb = const_tile([1, 128], F32, "onesrow")
            nc.gpsimd.dma_start(onesrow_sb[:], onesrow_d[:])
            one_sb = const_tile([1, 1], F32, "one")
            nc.gpsimd.dma_start(one_sb[:], one_d[:])
            ind_sb = const_tile([128, CK, G], F32, "ind")
            indt_sb = const_tile([G, CK, 128], F32, "indt")
            for ck in range(CK):
                nc.gpsimd.dma_start(ind_sb[:, ck, :], ind_d[ck])
                nc.gpsimd.dma_start(indt_sb[:, ck, :], indt_d[ck])
            # weights are not needed until the first QKV phase; load them
            # after the small consts so the identity (needed by the very
            # first transposes) isn't stuck behind the weight bytes
            for w_sb, w_d in ((wq_sb, wq_d), (wk_sb, wk_d), (wv_sb, wv_d),
                              (wp_sb, wp_d)):
                for ck in range(CK):
                    nc.gpsimd.dma_start(w_sb[:, ck, :], w_d[ck])
            # bp broadcast across partitions (rank-1 matmul) for the
            # bf16-residual path
            if use_bp:
                bpbc_sb = const_tile([128, C], F32, "bpbc")
                bp_ps = pm.tile([128, C], F32, tag="ps", bufs=2)
                nc.tensor.matmul(bp_ps[:], onesrow_sb[:], bp_sb[:],
                                 start=True, stop=True)
                nc.vector.tensor_copy(bpbc_sb[:], bp_ps[:])

            # ================= per-element phase emitters =================
            def emit_head(e):
                """P0-P2: load x, transpose, stats, normalize -> xn tiles."""
                hd = {}
                x_T = sb.tile([128, CK, S], BF16, tag="xT", bufs=1,
                              name="x_T")
                xp = sb.tile([128, CK, NT // 2], F32, tag="xp", bufs=2,
                             name="xp")
                for sp in range(NT // 2):
                    stages = []
                    for h in range(2):
                        st = 2 * sp + h
                        stage = sb.tile([128, C], F32, tag="xs", bufs=8,
                                        name="stage")
                        nc.sync.dma_start(stage[:],
                                          x_d[e, st * 128:(st + 1) * 128, :])
                        # bf16 copy so the PE transpose runs at 1 cyc/row
                        stage_b = sb.tile([128, C], BF16, tag="xsb", bufs=8,
                                          name="stage_b")
                        nc.vector.tensor_copy(stage_b[:], stage[:])
                        stages.append(stage_b)
                    for ck in range(CK):
                        tp = pm.tile([128, 2, 128], BF16, tag="ps", bufs=2,
                                     name="tp")
                        for h in range(2):
                            nc.tensor.matmul(
                                tp[:, h, :],
                                stages[h][:, ck * 128:(ck + 1) * 128],
                                identb_sb[:], is_transpose=True,
                                start=(h == 0), stop=(h == 1))
                        nc.vector.tensor_scalar(
                            x_T[:, ck, sp * 256:(sp + 1) * 256], tp[:],
                            0.0, None, op0=ALU.add, op1=ALU.add,
                            accum_out=xp[:, ck, sp:sp + 1])

                # group-norm statistics: sum from the evac accum, sum of
                # squares from two big ACT Square passes over x_T
                trash = sb.tile([128, S], FP8, tag="trash", bufs=1,
                                name="trash")
                st2s = []
                for ck in range(CK):
                    s2 = sb.tile([128, 2], F32, tag="st2", bufs=4, name="s2")
                    nc.vector.reduce_sum(s2[:, 0:1], xp[:, ck, :], axis=AX.X)
                    nc.scalar.activation(trash[:], x_T[:, ck, :], AF.Square,
                                         accum_out=s2[:, 1:2])
                    st2s.append(s2)
                gp = pm.tile([G, 2], F32, tag="ps", bufs=2, name="gp")
                for ck in range(CK):
                    nc.tensor.matmul(gp[:], ind_sb[:, ck, :], st2s[ck][:],
                                     start=(ck == 0), stop=(ck == CK - 1))
                m_e = sb.tile([G, 2], F32, tag="ge", bufs=4, name="m_e")
                nc.scalar.mul(m_e[:], gp[:], inv_n)
                mean2 = sb.tile([G, 1], F32, tag="ge1", bufs=6, name="mean2")
                nc.vector.tensor_mul(mean2[:], m_e[:, 0:1], m_e[:, 0:1])
                var = sb.tile([G, 1], F32, tag="ge1", bufs=6, name="var")
                nc.vector.tensor_sub(var[:], m_e[:, 1:2], mean2[:])
                nc.vector.tensor_scalar_add(var[:], var[:], EPS)
                std = sb.tile([G, 1], F32, tag="ge1", bufs=6, name="std")
                nc.scalar.activation(std[:], var[:], AF.Sqrt)
                mr = sb.tile([G, 2], F32, tag="ge", bufs=4, name="mr")
                nc.vector.tensor_copy(mr[:, 0:1], m_e[:, 0:1])
                nc.vector.reciprocal(mr[:, 1:2], std[:])

                # per-channel scale/bias; normalize x_T -> xn_b + xn8
                xn_b = sb.tile([128, CK, S], BF16, tag="xnb", bufs=1,
                               name="xn_b")
                xn8 = sb.tile([128, CK, S], FP8, tag="xn8", bufs=1,
                              name="xn8")
                for ck in range(CK):
                    mrc_ps = pm.tile([128, 2], F32, tag="ps", bufs=2,
                                     name="mrc_ps")
                    nc.tensor.matmul(mrc_ps[:], indt_sb[:, ck, :], mr[:],
                                     start=True, stop=True)
                    mrc = sb.tile([128, 2], F32, tag="st2", bufs=4, name="mrc")
                    nc.vector.tensor_copy(mrc[:], mrc_ps[:])
                    scale_c = sb.tile([128, 1], F32, tag="sc", bufs=8,
                                      name="scale_c")
                    nc.vector.tensor_mul(scale_c[:], mrc[:, 1:2],
                                         gamma_sb[:, ck:ck + 1])
                    t1 = sb.tile([128, 1], F32, tag="sc", bufs=8, name="t1")
                    nc.vector.tensor_mul(t1[:], mrc[:, 0:1], scale_c[:])
                    nb = sb.tile([128, 1], F32, tag="sc", bufs=8, name="nb")
                    nc.vector.tensor_sub(nb[:], beta_sb[:, ck:ck + 1], t1[:])
                    hd[("sc", ck)] = scale_c
                    hd[("nb", ck)] = nb
                # both normalize passes ride the otherwise idle GPSIMD
                # engine, keeping ACT free for the exp stream
                QS = max(S // 4, 512)
                for q0 in range(0, S, QS):
                    for ck in range(CK):
                        scale_c, nb = hd[("sc", ck)], hd[("nb", ck)]
                        nc.gpsimd.tensor_scalar(xn_b[:, ck, q0:q0 + QS],
                                                x_T[:, ck, q0:q0 + QS],
                                                scale_c[:], nb[:],
                                                op0=ALU.mult, op1=ALU.add)
                        nc.gpsimd.tensor_scalar(xn8[:, ck, q0:q0 + QS],
                                                x_T[:, ck, q0:q0 + QS],
                                                scale_c[:], nb[:],
                                                op0=ALU.mult, op1=ALU.add)
                hd["xn_b"] = xn_b
                hd["xn8"] = xn8
                return hd

            def emit_qkv(e, hd):
                """P3: q8/k8 (channel-major) and v8 (position-major), all
                fp8 via DoubleRow matmuls over the folded channel chunks."""
                xn8 = hd["xn8"]
                q_T = sb.tile([128, CK, S], FP8, tag="qT", bufs=1, name="q_T")
                k_T = sb.tile([128, CK, S], FP8, tag="kT", bufs=1, name="k_T")
                # PSUM evacuation is the qkv-phase bottleneck, so split it
                # between ACT (q, odd v tiles) and DVE (k, even v tiles)
                for w_sb, b_sb, out_t, on_act in (
                        (wq_sb, bq_sb, q_T, True), (wk_sb, bk_sb, k_T, False)):
                    for ct in range(CK):
                        for sbk in range(NSB):
                            ps = pm.tile([128, SB], F32, tag="pb", bufs=3,
                                         name="qk_ps")
                            nc.tensor.matmul(
                                ps[:],
                                w_sb[:, :, ct * 128:(ct + 1) * 128],
                                xn8[:, :, sbk * SB:(sbk + 1) * SB],
                                start=True, stop=True, perf_mode=DR)
                            dst = out_t[:, ct, sbk * SB:(sbk + 1) * SB]
                            if on_act:
                                nc.scalar.activation(
                                    dst, ps[:], AF.Identity,
                                    bias=b_sb[:, ct:ct + 1])
                            else:
                                nc.vector.tensor_scalar(
                                    dst, ps[:], b_sb[:, ct:ct + 1], None,
                                    op0=ALU.add)
                # v tiles: one DR matmul + one evac copy each
                v_sb = sb.tile([128, NT, C], FP8, tag="v", bufs=1, name="v_sb")
                for tt in range(NT):
                    ps = pm.tile([128, C], F32, tag="pb", bufs=3,
                                 name="v_ps")
                    nc.tensor.matmul(
                        ps[:],
                        xn8[:, :, tt * 128:(tt + 1) * 128],
                        wv_sb[:, :, :],
                        start=True, stop=(not use_bv), perf_mode=DR)
                    if use_bv:
                        nc.tensor.matmul(ps[:], onesrow_sb[:],
                                         bv_sb[:], start=False, stop=True)
                    if tt % 2 == 0:
                        nc.vector.tensor_copy(v_sb[:, tt, :], ps[:])
                    else:
                        nc.scalar.activation(v_sb[:, tt, :], ps[:],
                                             AF.Identity)
                hd["q_T"], hd["k_T"], hd["v_sb"] = q_T, k_T, v_sb
                hd["recip"] = sb.tile([128, NT], F32, tag="recip", bufs=2,
                                      name="recip")

            def emit_attention(e, hd, sbk):
                """P4 key-loop for one query block; returns finish closure."""
                q_T, k_T, v_sb = hd["q_T"], hd["k_T"], hd["v_sb"]
                recip_sb = hd["recip"]
                scol = slice(sbk * SB, (sbk + 1) * SB)
                exp_sb = sb.tile([128, NT, SB], FP8, tag="exp",
                                 bufs=exp_bufs, name="exp_sb")
                oU = pm.tile([128, CK, SB], F32, tag="accA", bufs=1,
                             name="oU")
                den_ps = pm.tile([1, SB], F32, tag="den", bufs=1,
                                 name="den_ps")

                def consume(p):
                    pcol = slice(2 * p, 2 * p + 2)
                    for ck in range(CK):
                        nc.tensor.matmul(
                            oU[:, ck, :],
                            v_sb[:, pcol, ck * 128:(ck + 1) * 128],
                            exp_sb[:, pcol, :],
                            start=(p == 0), stop=(p == NP - 1), perf_mode=DR)

                def den_mm(p):
                    nc.tensor.matmul(den_ps[:], ones8_sb[:, :, 0:1],
                                     exp_sb[:, 2 * p:2 * p + 2, :],
                                     start=(p == 0), stop=(p == NP - 1),
                                     perf_mode=DR)

                # one DR score matmul + one exp per key tile; the den
                # ones-matmul (per pair) and consume lag the exp so the
                # in-order PE queue never waits on ACT latency.
                for tt in range(NT):
                    ps_s = pm.tile([128, SB], F32, tag="pb", bufs=3,
                                   name="ps_s")
                    nc.tensor.matmul(
                        ps_s[:],
                        k_T[:, :, tt * 128:(tt + 1) * 128],
                        q_T[:, :, scol],
                        start=True, stop=True, perf_mode=DR)
                    nc.scalar.activation(exp_sb[:, tt, :], ps_s[:], AF.Exp,
                                         bias=expb_sb[:], scale=att_scale)
                    if tt % 2 == 1 and tt >= 3:
                        den_mm((tt - 3) // 2)
                    if tt % 2 == 1 and tt >= 7:
                        consume((tt - 7) // 2)
                den_mm(NP - 1)
                for p in range(NP - 3, NP):
                    consume(p)

                def evac():
                    oU_sb = sb.tile([128, CK, SB], BF16, tag="oU", bufs=2,
                                    name="oU_sb")
                    nc.vector.tensor_copy(oU_sb[:], oU[:])
                    den_sb = sb.tile([1, SB], F32, tag="denc", bufs=2,
                                     name="den_sb")
                    nc.vector.tensor_copy(den_sb[:], den_ps[:])
                    return oU_sb, den_sb

                def finish(ev):
                    oU_sb, den_sb = ev
                    # denominator -> per-partition reciprocal
                    dT_ps = pm.tile([128, SB // 128], F32, tag="ps", bufs=2,
                                    name="dT_ps")
                    for j in range(SB // 128):
                        nc.tensor.matmul(dT_ps[:, j:j + 1],
                                         den_sb[0:1, j * 128:(j + 1) * 128],
                                         one_sb[:], start=(j == 0),
                                         stop=(j == SB // 128 - 1))
                    nc.vector.reciprocal(
                        recip_sb[:, sbk * (SB // 128):(sbk + 1) * (SB // 128)],
                        dT_ps[:])

                    # projection + residual + output
                    xn_b = hd["xn_b"]
                    for st in range(SB // 128):
                        gst = sbk * (SB // 128) + st
                        prj = pm.tile([128, C], F32, tag="ps", bufs=2,
                                      name="prj")
                        for ck in range(CK):
                            nc.tensor.matmul(
                                prj[:], oU_sb[:, ck, st * 128:(st + 1) * 128],
                                wp_sb[:, ck, :],
                                start=(ck == 0), stop=(ck == CK - 1))
                        res = pm.tile([128, C], BF16, tag="ps", bufs=2,
                                      name="res")
                        for ck in range(CK):
                            nc.tensor.matmul(
                                res[:, ck * 128:(ck + 1) * 128],
                                xn_b[:, ck, gst * 128:(gst + 1) * 128],
                                identb_sb[:], is_transpose=True,
                                start=(ck == 0), stop=(ck == CK - 1))
                        out_sb = sb.tile([128, C], F32, tag="out", bufs=3,
                                         name="out_sb")
                        nc.vector.tensor_scalar(out_sb[:], prj[:],
                                                recip_sb[:, gst:gst + 1],
                                                None, op0=ALU.mult)
                        nc.vector.tensor_add(out_sb[:], out_sb[:], res[:])
                        if use_bp:
                            nc.vector.tensor_add(out_sb[:], out_sb[:],
                                                 bpbc_sb[:])
                        nc.sync.dma_start(
                            y_d[e, gst * 128:(gst + 1) * 128, :], out_sb[:])

                return evac, finish

            # ============ software pipeline across batch elements ============
            hd = emit_head(0)
            emit_qkv(0, hd)
            prev = None  # (evac_fn, finish_fn) of previous query block
            for e in range(B_loc):
                nxt = None
                for sbk in range(NSB):
                    if prev is not None:
                        prev_ev = prev[0]()   # evacuate previous block's oU
                    ev_fn, fin_fn = emit_attention(e, hd, sbk)
                    if prev is not None:
                        prev[1](prev_ev)      # finish previous block
                    prev = (ev_fn, fin_fn)
                    if e + 1 < B_loc and sbk == max(0, NSB - 3):
                        nxt = emit_head(e + 1)
                if e + 1 < B_loc:
                    if nxt is None:
                        nxt = emit_head(e + 1)
                    prev[1](prev[0]())
                    prev = None
                    emit_qkv(e + 1, nxt)
                    hd = nxt
            if prev is not None:
                prev[1](prev[0]())

    return nc


def make_const_inputs(C=256, G=8):
    """Host-side constant arrays shared by all cores."""
    CK = C // 128
    cpg = C // G            # channels per group (32)
    gpc = 128 // cpg        # groups per chunk (4)
    ind = np.zeros((CK, 128, G), np.float32)
    indt = np.zeros((CK, G, 128), np.float32)
    for ck in range(CK):
        for p in range(128):
            g = ck * gpc + p // cpg
            ind[ck, p, g] = 1.0
            indt[ck, g, p] = 1.0
    return {
        "ident": np.eye(128, dtype=np.float32),
        "identb": np.eye(128, dtype=np.float32).astype(ml_dtypes.bfloat16),
        "ones8": np.ones((128, 2, 16), ml_dtypes.float8_e4m3),
        "expb": np.full((128, 1), EXP_BIAS, np.float32),
        "onesrow": np.ones((1, 128), np.float32),
        "one": np.ones((1, 1), np.float32),
        "ind": ind,
        "indt": indt,
    }


def make_weight_inputs(Wq, bq, Wk, bk, Wv, bv, Wp, bp, gamma, beta):
    C = Wq.shape[0]
    CK = C // 128

    def wchunk8(w):
        w = np.clip(np.asarray(w, np.float32), -240.0, 240.0)
        return np.ascontiguousarray(w.reshape(CK, 128, C)).astype(
            ml_dtypes.float8_e4m3)

    def wchunkb(w):
        return np.ascontiguousarray(
            np.asarray(w, np.float32).reshape(CK, 128, C)).astype(
                ml_dtypes.bfloat16)

    def pcol(v):
        return np.ascontiguousarray(
            np.asarray(v, np.float32).reshape(CK, 128, 1))

    def row(v):
        return np.ascontiguousarray(np.asarray(v, np.float32).reshape(1, C))

    return {
        "wq": wchunk8(Wq), "wk": wchunk8(Wk), "wv": wchunk8(Wv),
        "wp": wchunkb(Wp),
        "bq": pcol(bq), "bk": pcol(bk), "bv": row(bv), "bp": row(bp),
        "gamma": pcol(gamma), "beta": pcol(beta),
    }


_NC_CACHE = {}


def _get_compiled_nc(B_loc, S, C, use_bv=True, use_bp=True):
    key = (B_loc, S, C, use_bv, use_bp)
    if key not in _NC_CACHE:
        nc = build_nc(B_loc=B_loc, S=S, C=C, use_bv=use_bv, use_bp=use_bp)
        nc.finalize()
        _NC_CACHE[key] = nc
    return _NC_CACHE[key]


def _make_runner(nc, n_cores):
    """Build a reusable jitted SPMD executable (same lowering path as
    concourse.bass2jax.run_bass_via_pjrt, kept so repeat kernel() calls
    skip retracing/recompiling)."""
    import jax
    from jax.sharding import Mesh, PartitionSpec, NamedSharding
    from jax.experimental.shard_map import shard_map
    from concourse import bass2jax

    bass2jax.install_neuronx_cc_hook()
    partition_name = (nc.partition_id_tensor.name
                      if nc.partition_id_tensor else None)
    in_names, out_names, out_avals, zero_outs = [], [], [], []
    for alloc in nc.m.functions[0].allocations:
        if not isinstance(alloc, mybir.MemoryLocationSet):
            continue
        name = alloc.memorylocations[0].name
        if alloc.kind == "ExternalInput":
            if name != partition_name:
                in_names.append(name)
        elif alloc.kind == "ExternalOutput":
            shape = tuple(alloc.tensor_shape)
            dtype = mybir.dt.np(alloc.dtype)
            out_avals.append(jax.core.ShapedArray(shape, dtype))
            out_names.append(name)
            zero_outs.append(np.zeros(shape, dtype))
    n_params = len(in_names)
    all_names = in_names + out_names
    if partition_name is not None:
        all_names = all_names + [partition_name]

    def _body(*args):
        operands = list(args)
        if partition_name is not None:
            operands.append(bass2jax.partition_id_tensor())
        outs = bass2jax._bass_exec_p.bind(
            *operands, out_avals=tuple(out_avals),
            in_names=tuple(all_names), out_names=tuple(out_names),
            lowering_input_output_aliases=(),
            sim_require_finite=True, sim_require_nnan=True, nc=nc)
        return tuple(outs)

    devices = jax.devices()[:n_cores]
    mesh = Mesh(np.asarray(devices), ("core",))
    specs = (PartitionSpec("core"),) * (n_params + len(out_names))
    fn = jax.jit(shard_map(_body, mesh=mesh, in_specs=specs,
                           out_specs=(PartitionSpec("core"),) * len(out_names),
                           check_rep=False), keep_unused=True)
    sh = NamedSharding(mesh, PartitionSpec("core"))

    def run(in_maps):
        concat_in = [np.concatenate([np.asarray(m[n]) for m in in_maps],
                                    axis=0) for n in in_names]
        concat_zeros = [np.zeros((n_cores * z.shape[0], *z.shape[1:]),
                                 z.dtype) for z in zero_outs]
        outs = fn(*[jax.device_put(a, sh) for a in concat_in],
                  *[jax.device_put(z, sh) for z in concat_zeros])
        return {name: np.asarray(outs[i])
                for i, name in enumerate(out_names)}

    return run


_RUNNER_CACHE = {}


def kernel(x, gamma, beta, Wq, bq, Wk, bk, Wv, bv, Wp, bp):
    x = np.asarray(x, np.float32)
    B, H, W, C = x.shape
    S = H * W
    assert B % N_CORES == 0
    B_loc = B // N_CORES

    use_bv = bool(np.any(np.asarray(bv)))
    use_bp = bool(np.any(np.asarray(bp)))
    key = (B_loc, S, C, use_bv, use_bp)
    if key not in _RUNNER_CACHE:
        nc = _get_compiled_nc(B_loc, S, C, use_bv, use_bp)
        _RUNNER_CACHE[key] = _make_runner(nc, N_CORES)
    run = _RUNNER_CACHE[key]

    shared = make_const_inputs(C=C)
    shared.update(make_weight_inputs(Wq, bq, Wk, bk, Wv, bv, Wp, bp,
                                     gamma, beta))
    xr = x.reshape(B, S, C)
    in_maps = [
        {**shared, "x": np.ascontiguousarray(xr[k * B_loc:(k + 1) * B_loc])}
        for k in range(N_CORES)
    ]
    out = run(in_maps)
    y = out["y"].reshape(N_CORES, B_loc, S, C).reshape(B, S, C)
    return np.ascontiguousarray(y.reshape(B, H, W, C).astype(np.float32))


# revision 16
# speedup vs baseline: 1.0952x; 1.0952x over previous
"""Trainium2 Bass kernel for an AttentionBlock (GroupNorm + single-head
self-attention + projection + residual), data-parallel over batch on 8
NeuronCores.

Reference computation (per batch element, S = H*W = 4096, C = 256):
    xn   = GroupNorm(x, groups=8, eps=1e-3) * gamma + beta
    q    = xn @ Wq + bq ; k = xn @ Wk + bk ; v = xn @ Wv + bv
    attn = softmax((q @ k^T) / sqrt(C))
    out  = attn @ v
    y    = xn + (out @ Wp + bp)

Layout strategy (per core, B/8 batch elements, software-pipelined):
  - x tiles are DMA-loaded in [s, c] layout, converted to bf16 on DVE
    and transposed to x_T [c, s] by the DMA XBAR (dma_start_transpose):
    the PE does no transpose work at all.
  - GroupNorm stats: per-channel sums via a DVE reduce, sums of squares
    via two big ACT Square passes (accum_out), then two tiny
    cross-partition matmuls with group indicator matrices. The per
    channel normalize runs on the otherwise idle GPSIMD engine,
    producing xn_b (bf16, residual) and xn8 (fp8e4, QKV).
  - All attention-sized matmuls (QKV, scores, attn@v) are fp8e4
    DoubleRow matmuls (two 128-deep contraction chunks folded into one
    PE pass). q8/k8 in [c, s], v8 in [t, c], exp8 in [t, s].
  - softmax skips max-subtraction but shifts exp by -3*ln2 so the
    largest exp stays below fp8e4's 240 limit (the shift cancels in the
    numerator/denominator ratio). exp is written fp8 directly, one ACT
    instruction per PSUM pair (FD=1024) to amortize ACT fixed overhead.
  - The denominator is one DoubleRow ones-matmul per key-tile pair into
    a persistent [1, SB] PSUM; its reciprocal is broadcast across
    partitions (XBAR) and applied during the oU evacuation, so the
    projection PSUM can accumulate projection + residual (bf16
    identity-matmul transpose) + bias in one group and evacuate with a
    single copy.
  - Per query block, the finish steps are spread across the NEXT
    block's key loop so the single projection PSUM slot never stalls
    the in-order PE queue; the next element's head load runs during the
    current element's late blocks, with its stats deferred two blocks
    so the tiny stats matmuls never wait in the PE queue.
"""

import math
import os
import sys

for _p in ("/opt/trn_rl_repo", "/root/.axon_site/_ro/trn_rl_repo"):
    if os.path.isdir(_p) and _p not in sys.path:
        sys.path.append(_p)

import numpy as np
import ml_dtypes

import concourse.bass as bass
import concourse.mybir as mybir
import concourse.tile as tile
from concourse import bacc

F32 = mybir.dt.float32
BF16 = mybir.dt.bfloat16
FP8 = mybir.dt.float8e4
AF = mybir.ActivationFunctionType
AX = mybir.AxisListType
ALU = mybir.AluOpType
DR = mybir.MatmulPerfMode.DoubleRow

N_CORES = 8
EXP_BIAS = -3.0 * math.log(2.0)  # keep exp() under fp8e4 max (240)


def build_nc(B_loc=2, S=4096, C=256, G=8, EPS=1e-3, exp_bufs=2,
             use_bv=True, use_bp=True):
    """Build the single-core Bass program (SPMD: same program all cores)."""
    nc = bacc.Bacc(None, target_bir_lowering=False, debug=False)

    CK = C // 128          # channel chunks (2)
    NT = S // 128          # key tiles (32)
    NP = NT // 2           # key tile pairs (16)
    SB = 512               # query block size
    NSB = S // SB          # query blocks (8)
    att_scale = float(C) ** -0.5
    inv_n = 1.0 / float(S * (C // G))

    x_d = nc.dram_tensor("x", [B_loc, S, C], F32, kind="ExternalInput")
    y_d = nc.dram_tensor("y", [B_loc, S, C], F32, kind="ExternalOutput")
    wq_d = nc.dram_tensor("wq", [CK, 128, C], FP8, kind="ExternalInput")
    wk_d = nc.dram_tensor("wk", [CK, 128, C], FP8, kind="ExternalInput")
    wv_d = nc.dram_tensor("wv", [CK, 128, C], FP8, kind="ExternalInput")
    wp_d = nc.dram_tensor("wp", [CK, 128, C], BF16, kind="ExternalInput")
    bq_d = nc.dram_tensor("bq", [CK, 128, 1], F32, kind="ExternalInput")
    bk_d = nc.dram_tensor("bk", [CK, 128, 1], F32, kind="ExternalInput")
    bvb_d = nc.dram_tensor("bvb", [1, C], BF16, kind="ExternalInput")
    bpb_d = nc.dram_tensor("bpb", [1, C], BF16, kind="ExternalInput")
    gamma_d = nc.dram_tensor("gamma", [CK, 128, 1], F32, kind="ExternalInput")
    beta_d = nc.dram_tensor("beta", [CK, 128, 1], F32, kind="ExternalInput")
    identb_d = nc.dram_tensor("identb", [128, 128], BF16, kind="ExternalInput")
    ones8_d = nc.dram_tensor("ones8", [128, 2, 16], FP8, kind="ExternalInput")
    expb_d = nc.dram_tensor("expb", [128, 1], F32, kind="ExternalInput")
    onesrowb_d = nc.dram_tensor("onesrowb", [1, 128], BF16,
                                kind="ExternalInput")
    ind_d = nc.dram_tensor("ind", [CK, 128, G], F32, kind="ExternalInput")
    indt_d = nc.dram_tensor("indt", [CK, G, 128], F32, kind="ExternalInput")

    with tile.TileContext(nc) as tc:
        with (
            tc.tile_pool(name="sb", bufs=1) as sb,
            tc.tile_pool(name="pm", bufs=1, space="PSUM") as pm,
        ):
            # ---- load constants (resident for the whole kernel) ----
            def const_tile(shape, dtype, tag):
                return sb.tile(shape, dtype, tag=tag, bufs=1, name=tag)

            wq_sb = const_tile([128, CK, C], FP8, "wq")
            wk_sb = const_tile([128, CK, C], FP8, "wk")
            wv_sb = const_tile([128, CK, C], FP8, "wv")
            wp_sb = const_tile([128, CK, C], BF16, "wp")
            bq_sb = const_tile([128, CK], F32, "bq")
            bk_sb = const_tile([128, CK], F32, "bk")
            gamma_sb = const_tile([128, CK], F32, "gamma")
            beta_sb = const_tile([128, CK], F32, "beta")
            for b_sb, b_d in ((bq_sb, bq_d), (bk_sb, bk_d),
                              (gamma_sb, gamma_d), (beta_sb, beta_d)):
                for ck in range(CK):
                    nc.gpsimd.dma_start(b_sb[:, ck:ck + 1], b_d[ck])
            bvb_sb = const_tile([1, C], BF16, "bvb")
            bpb_sb = const_tile([1, C], BF16, "bpb")
            nc.gpsimd.dma_start(bvb_sb[:], bvb_d[:])
            nc.gpsimd.dma_start(bpb_sb[:], bpb_d[:])
            identb_sb = const_tile([128, 128], BF16, "identb")
            nc.gpsimd.dma_start(identb_sb[:], identb_d[:])
            ones8_sb = const_tile([128, 2, 16], FP8, "ones8")
            nc.gpsimd.dma_start(ones8_sb[:], ones8_d[:])
            expb_sb = const_tile([128, 1], F32, "expb")
            nc.gpsimd.dma_start(expb_sb[:], expb_d[:])
            onesrowb_sb = const_tile([1, 128], BF16, "onesrowb")
            nc.gpsimd.dma_start(onesrowb_sb[:], onesrowb_d[:])
            ind_sb = const_tile([128, CK, G], F32, "ind")
            indt_sb = const_tile([G, CK, 128], F32, "indt")
            for ck in range(CK):
                nc.gpsimd.dma_start(ind_sb[:, ck, :], ind_d[ck])
                nc.gpsimd.dma_start(indt_sb[:, ck, :], indt_d[ck])
            for w_sb, w_d in ((wq_sb, wq_d), (wk_sb, wk_d), (wv_sb, wv_d),
                              (wp_sb, wp_d)):
                for ck in range(CK):
                    nc.gpsimd.dma_start(w_sb[:, ck, :], w_d[ck])

            # ================= per-element phase emitters =================
            def emit_head_load(e):
                """Load x, bf16-convert, XBAR-transpose to x_T [c, s].
                No PE work at all."""
                hd = {}
                x_T = sb.tile([128, CK, S], BF16, tag="xT", bufs=1,
                              name="x_T")
                for st in range(NT):
                    stage = sb.tile([128, C], F32, tag="xs", bufs=8,
                                    name="stage")
                    nc.sync.dma_start(stage[:],
                                      x_d[e, st * 128:(st + 1) * 128, :])
                    stage_b = sb.tile([128, C], BF16, tag="xsb", bufs=8,
                                      name="stage_b")
                    nc.vector.tensor_copy(stage_b[:], stage[:])
                    for ck in range(CK):
                        nc.sync.dma_start_transpose(
                            x_T[:, ck, st * 128:(st + 1) * 128],
                            stage_b[:, ck * 128:(ck + 1) * 128])
                hd["x_T"] = x_T
                return hd

            def emit_head_stats(e, hd):
                """Stats + normalize: mostly ACT/DVE/GPSIMD; only two tiny
                PE matmuls (emitted well after their inputs are ready)."""
                x_T = hd["x_T"]
                trash = sb.tile([128, S], FP8, tag="trash", bufs=1,
                                name="trash")
                st2s = []
                for ck in range(CK):
                    s2 = sb.tile([128, 2], F32, tag="st2", bufs=4, name="s2")
                    nc.vector.reduce_sum(s2[:, 0:1], x_T[:, ck, :], axis=AX.X)
                    nc.scalar.activation(trash[:], x_T[:, ck, :], AF.Square,
                                         accum_out=s2[:, 1:2])
                    st2s.append(s2)
                gp = pm.tile([G, 2], F32, tag="ps", bufs=1, name="gp")
                for ck in range(CK):
                    nc.tensor.matmul(gp[:], ind_sb[:, ck, :], st2s[ck][:],
                                     start=(ck == 0), stop=(ck == CK - 1))
                m_e = sb.tile([G, 2], F32, tag="ge", bufs=4, name="m_e")
                nc.scalar.mul(m_e[:], gp[:], inv_n)
                mean2 = sb.tile([G, 1], F32, tag="ge1", bufs=6, name="mean2")
                nc.vector.tensor_mul(mean2[:], m_e[:, 0:1], m_e[:, 0:1])
                var = sb.tile([G, 1], F32, tag="ge1", bufs=6, name="var")
                nc.vector.tensor_sub(var[:], m_e[:, 1:2], mean2[:])
                nc.vector.tensor_scalar_add(var[:], var[:], EPS)
                std = sb.tile([G, 1], F32, tag="ge1", bufs=6, name="std")
                nc.scalar.activation(std[:], var[:], AF.Sqrt)
                mr = sb.tile([G, 2], F32, tag="ge", bufs=4, name="mr")
                nc.vector.tensor_copy(mr[:, 0:1], m_e[:, 0:1])
                nc.vector.reciprocal(mr[:, 1:2], std[:])

                xn_b = sb.tile([128, CK, S], BF16, tag="xnb", bufs=1,
                               name="xn_b")
                xn8 = sb.tile([128, CK, S], FP8, tag="xn8", bufs=1,
                              name="xn8")
                for ck in range(CK):
                    mrc_ps = pm.tile([128, 2], F32, tag="ps", bufs=1,
                                     name="mrc_ps")
                    nc.tensor.matmul(mrc_ps[:], indt_sb[:, ck, :], mr[:],
                                     start=True, stop=True)
                    mrc = sb.tile([128, 2], F32, tag="st2", bufs=4, name="mrc")
                    nc.vector.tensor_copy(mrc[:], mrc_ps[:])
                    scale_c = sb.tile([128, 1], F32, tag="sc", bufs=8,
                                      name="scale_c")
                    nc.vector.tensor_mul(scale_c[:], mrc[:, 1:2],
                                         gamma_sb[:, ck:ck + 1])
                    t1 = sb.tile([128, 1], F32, tag="sc", bufs=8, name="t1")
                    nc.vector.tensor_mul(t1[:], mrc[:, 0:1], scale_c[:])
                    nb = sb.tile([128, 1], F32, tag="sc", bufs=8, name="nb")
                    nc.vector.tensor_sub(nb[:], beta_sb[:, ck:ck + 1], t1[:])
                    hd[("sc", ck)] = scale_c
                    hd[("nb", ck)] = nb
                # both normalize passes ride the otherwise idle GPSIMD
                QS = max(S // 4, 512)
                for q0 in range(0, S, QS):
                    for ck in range(CK):
                        scale_c, nb = hd[("sc", ck)], hd[("nb", ck)]
                        nc.gpsimd.tensor_scalar(xn_b[:, ck, q0:q0 + QS],
                                                x_T[:, ck, q0:q0 + QS],
                                                scale_c[:], nb[:],
                                                op0=ALU.mult, op1=ALU.add)
                        nc.gpsimd.tensor_scalar(xn8[:, ck, q0:q0 + QS],
                                                x_T[:, ck, q0:q0 + QS],
                                                scale_c[:], nb[:],
                                                op0=ALU.mult, op1=ALU.add)
                hd["xn_b"] = xn_b
                hd["xn8"] = xn8

            def emit_qkv(e, hd):
                """q8/k8 (channel-major) and v8 (position-major), all fp8
                DoubleRow; evacuation split between ACT and DVE."""
                xn8 = hd["xn8"]
                q_T = sb.tile([128, CK, S], FP8, tag="qT", bufs=1, name="q_T")
                k_T = sb.tile([128, CK, S], FP8, tag="kT", bufs=1, name="k_T")
                for w_sb, b_sb, out_t, on_act in (
                        (wq_sb, bq_sb, q_T, True), (wk_sb, bk_sb, k_T, False)):
                    for ct in range(CK):
                        for pr in range(NSB // 2):
                            ps = pm.tile([128, 2, SB], F32, tag="pb", bufs=2,
                                         name="qk_ps")
                            for h in range(2):
                                sbk = 2 * pr + h
                                nc.tensor.matmul(
                                    ps[:, h, :],
                                    w_sb[:, :, ct * 128:(ct + 1) * 128],
                                    xn8[:, :, sbk * SB:(sbk + 1) * SB],
                                    start=True, stop=True, perf_mode=DR)
                            dst = out_t[:, ct,
                                        2 * pr * SB:(2 * pr + 2) * SB]
                            if on_act:
                                nc.scalar.activation(
                                    dst, ps[:], AF.Identity,
                                    bias=b_sb[:, ct:ct + 1])
                            else:
                                nc.vector.tensor_scalar(
                                    dst, ps[:], b_sb[:, ct:ct + 1], None,
                                    op0=ALU.add)
                v_sb = sb.tile([128, NT, C], FP8, tag="v", bufs=1, name="v_sb")
                for tv in range(NT // 2):
                    ps = pm.tile([128, 2, C], F32, tag="pb", bufs=2,
                                 name="v_ps")
                    for h in range(2):
                        tt = 2 * tv + h
                        nc.tensor.matmul(
                            ps[:, h, :],
                            xn8[:, :, tt * 128:(tt + 1) * 128],
                            wv_sb[:, :, :],
                            start=True, stop=(not use_bv), perf_mode=DR)
                        if use_bv:
                            nc.tensor.matmul(ps[:, h, :], onesrowb_sb[:],
                                             bvb_sb[:], start=False,
                                             stop=True)
                    if tv % 2 == 0:
                        nc.vector.tensor_copy(v_sb[:, 2 * tv:2 * tv + 2, :],
                                              ps[:])
                    else:
                        nc.scalar.activation(v_sb[:, 2 * tv:2 * tv + 2, :],
                                             ps[:], AF.Identity)
                hd["q_T"], hd["k_T"], hd["v_sb"] = q_T, k_T, v_sb

            def emit_attention(e, hd, sbk, pending):
                """Key loop for one query block; `pending` holds the
                previous block's finish steps, invoked at fixed pair
                indices so their PSUM slot reuse never stalls the PE."""
                q_T, k_T, v_sb = hd["q_T"], hd["k_T"], hd["v_sb"]
                scol = slice(sbk * SB, (sbk + 1) * SB)
                exp_sb = sb.tile([128, NT, SB], FP8, tag="exp",
                                 bufs=exp_bufs, name="exp_sb")
                oU = pm.tile([128, CK, SB], F32, tag="accA", bufs=1,
                             name="oU")
                den_ps = pm.tile([1, SB], F32, tag="den", bufs=1,
                                 name="den_ps")

                def consume(p):
                    pcol = slice(2 * p, 2 * p + 2)
                    for ck in range(CK):
                        nc.tensor.matmul(
                            oU[:, ck, :],
                            v_sb[:, pcol, ck * 128:(ck + 1) * 128],
                            exp_sb[:, pcol, :],
                            start=(p == 0), stop=(p == NP - 1), perf_mode=DR)

                def den_mm(p):
                    nc.tensor.matmul(den_ps[:], ones8_sb[:, :, 0:1],
                                     exp_sb[:, 2 * p:2 * p + 2, :],
                                     start=(p == 0), stop=(p == NP - 1),
                                     perf_mode=DR)

                step_at = {5, 8, 11, 14}
                for tp_i in range(NP):
                    ps_s = pm.tile([128, 2, SB], F32, tag="pb", bufs=2,
                                   name="ps_s")
                    for h in range(2):
                        tt = 2 * tp_i + h
                        nc.tensor.matmul(
                            ps_s[:, h, :],
                            k_T[:, :, tt * 128:(tt + 1) * 128],
                            q_T[:, :, scol],
                            start=True, stop=True, perf_mode=DR)
                    pcol = slice(2 * tp_i, 2 * tp_i + 2)
                    nc.scalar.activation(exp_sb[:, pcol, :], ps_s[:], AF.Exp,
                                         bias=expb_sb[:], scale=att_scale)
                    if tp_i >= 1:
                        den_mm(tp_i - 1)
                    if tp_i > 2:
                        consume(tp_i - 3)
                    if pending and tp_i in step_at:
                        pending.pop(0)()
                den_mm(NP - 1)
                for p in range(NP - 3, NP):
                    consume(p)

                def evac():
                    # denominator reciprocal, broadcast, and normalization
                    # applied during the oU evacuation
                    den_sb = sb.tile([1, SB], F32, tag="denc", bufs=2,
                                     name="den_sb")
                    nc.vector.tensor_copy(den_sb[:], den_ps[:])
                    den_inv = sb.tile([1, SB], F32, tag="deni", bufs=2,
                                      name="den_inv")
                    nc.vector.reciprocal(den_inv[:], den_sb[:])
                    den_bc = sb.tile([128, SB], F32, tag="denbc", bufs=2,
                                     name="den_bc")
                    nc.gpsimd.partition_broadcast(den_bc[:], den_inv[:])
                    oU_sb = sb.tile([128, CK, SB], BF16, tag="oU", bufs=2,
                                    name="oU_sb")
                    for ck in range(CK):
                        nc.vector.tensor_tensor(oU_sb[:, ck, :],
                                                oU[:, ck, :], den_bc[:],
                                                op=ALU.mult)
                    return oU_sb

                def make_steps(oU_sb):
                    xn_b = hd["xn_b"]

                    def step(st):
                        def go():
                            gst = sbk * (SB // 128) + st
                            prj = pm.tile([128, C], F32, tag="ps", bufs=1,
                                          name="prj")
                            n_mm = 2 * CK + (1 if use_bp else 0)
                            i = 0
                            for ck in range(CK):
                                nc.tensor.matmul(
                                    prj[:],
                                    oU_sb[:, ck, st * 128:(st + 1) * 128],
                                    wp_sb[:, ck, :],
                                    start=(i == 0), stop=(i == n_mm - 1))
                                i += 1
                            # residual: bf16 identity matmul accumulates the
                            # transpose of xn_b straight into the projection
                            for ck in range(CK):
                                nc.tensor.matmul(
                                    prj[:, ck * 128:(ck + 1) * 128],
                                    xn_b[:, ck, gst * 128:(gst + 1) * 128],
                                    identb_sb[:],
                                    start=(i == 0), stop=(i == n_mm - 1))
                                i += 1
                            if use_bp:
                                nc.tensor.matmul(prj[:], onesrowb_sb[:],
                                                 bpb_sb[:], start=False,
                                                 stop=True)
                            out_sb = sb.tile([128, C], F32, tag="out",
                                             bufs=3, name="out_sb")
                            nc.vector.tensor_copy(out_sb[:], prj[:])
                            nc.sync.dma_start(
                                y_d[e, gst * 128:(gst + 1) * 128, :],
                                out_sb[:])
                        return go

                    return [step(st) for st in range(SB // 128)]

                return evac, make_steps

            # ============ software pipeline across batch elements ============
            hd = emit_head_load(0)
            emit_head_stats(0, hd)
            emit_qkv(0, hd)
            prev = None  # (evac_fn, make_steps_fn) of previous query block
            for e in range(B_loc):
                nxt = None
                for sbk in range(NSB):
                    steps = []
                    if prev is not None:
                        oU_sb = prev[0]()
                        steps = prev[1](oU_sb)
                    prev = emit_attention(e, hd, sbk, steps)
                    assert not steps, "unconsumed finish steps"
                    if e + 1 < B_loc and sbk == NSB - 4:
                        nxt = emit_head_load(e + 1)
                    if e + 1 < B_loc and sbk == NSB - 2:
                        emit_head_stats(e + 1, nxt)
                if e + 1 < B_loc:
                    oU_sb = prev[0]()
                    for s in prev[1](oU_sb):
                        s()
                    prev = None
                    emit_qkv(e + 1, nxt)
                    hd = nxt
            if prev is not None:
                oU_sb = prev[0]()
                for s in prev[1](oU_sb):
                    s()

    return nc


def make_const_inputs(C=256, G=8):
    """Host-side constant arrays shared by all cores."""
    CK = C // 128
    cpg = C // G            # channels per group (32)
    gpc = 128 // cpg        # groups per chunk (4)
    ind = np.zeros((CK, 128, G), np.float32)
    indt = np.zeros((CK, G, 128), np.float32)
    for ck in range(CK):
        for p in range(128):
            g = ck * gpc + p // cpg
            ind[ck, p, g] = 1.0
            indt[ck, g, p] = 1.0
    return {
        "identb": np.eye(128, dtype=np.float32).astype(ml_dtypes.bfloat16),
        "ones8": np.ones((128, 2, 16), ml_dtypes.float8_e4m3),
        "expb": np.full((128, 1), EXP_BIAS, np.float32),
        "onesrowb": np.ones((1, 128), ml_dtypes.bfloat16),
        "ind": ind,
        "indt": indt,
    }


def make_weight_inputs(Wq, bq, Wk, bk, Wv, bv, Wp, bp, gamma, beta):
    C = Wq.shape[0]
    CK = C // 128

    def wchunk8(w):
        w = np.clip(np.asarray(w, np.float32), -240.0, 240.0)
        return np.ascontiguousarray(w.reshape(CK, 128, C)).astype(
            ml_dtypes.float8_e4m3)

    def wchunkb(w):
        return np.ascontiguousarray(
            np.asarray(w, np.float32).reshape(CK, 128, C)).astype(
                ml_dtypes.bfloat16)

    def pcol(v):
        return np.ascontiguousarray(
            np.asarray(v, np.float32).reshape(CK, 128, 1))

    def rowb(v):
        return np.ascontiguousarray(
            np.asarray(v, np.float32).reshape(1, C)).astype(
                ml_dtypes.bfloat16)

    return {
        "wq": wchunk8(Wq), "wk": wchunk8(Wk), "wv": wchunk8(Wv),
        "wp": wchunkb(Wp),
        "bq": pcol(bq), "bk": pcol(bk), "bvb": rowb(bv), "bpb": rowb(bp),
        "gamma": pcol(gamma), "beta": pcol(beta),
    }


_NC_CACHE = {}


def _get_compiled_nc(B_loc, S, C, use_bv=True, use_bp=True):
    key = (B_loc, S, C, use_bv, use_bp)
    if key not in _NC_CACHE:
        nc = build_nc(B_loc=B_loc, S=S, C=C, use_bv=use_bv, use_bp=use_bp)
        nc.finalize()
        _NC_CACHE[key] = nc
    return _NC_CACHE[key]


def _make_runner(nc, n_cores):
    """Build a reusable jitted SPMD executable (same lowering path as
    concourse.bass2jax.run_bass_via_pjrt, kept so repeat kernel() calls
    skip retracing/recompiling)."""
    import jax
    from jax.sharding import Mesh, PartitionSpec, NamedSharding
    from jax.experimental.shard_map import shard_map
    from concourse import bass2jax

    bass2jax.install_neuronx_cc_hook()
    partition_name = (nc.partition_id_tensor.name
                      if nc.partition_id_tensor else None)
    in_names, out_names, out_avals, zero_outs = [], [], [], []
    for alloc in nc.m.functions[0].allocations:
        if not isinstance(alloc, mybir.MemoryLocationSet):
            continue
        name = alloc.memorylocations[0].name
        if alloc.kind == "ExternalInput":
            if name != partition_name:
                in_names.append(name)
        elif alloc.kind == "ExternalOutput":
            shape = tuple(alloc.tensor_shape)
            dtype = mybir.dt.np(alloc.dtype)
            out_avals.append(jax.core.ShapedArray(shape, dtype))
            out_names.append(name)
            zero_outs.append(np.zeros(shape, dtype))
    n_params = len(in_names)
    all_names = in_names + out_names
    if partition_name is not None:
        all_names = all_names + [partition_name]

    def _body(*args):
        operands = list(args)
        if partition_name is not None:
            operands.append(bass2jax.partition_id_tensor())
        outs = bass2jax._bass_exec_p.bind(
            *operands, out_avals=tuple(out_avals),
            in_names=tuple(all_names), out_names=tuple(out_names),
            lowering_input_output_aliases=(),
            sim_require_finite=True, sim_require_nnan=True, nc=nc)
        return tuple(outs)

    devices = jax.devices()[:n_cores]
    mesh = Mesh(np.asarray(devices), ("core",))
    specs = (PartitionSpec("core"),) * (n_params + len(out_names))
    fn = jax.jit(shard_map(_body, mesh=mesh, in_specs=specs,
                           out_specs=(PartitionSpec("core"),) * len(out_names),
                           check_rep=False), keep_unused=True)
    sh = NamedSharding(mesh, PartitionSpec("core"))

    def run(in_maps):
        concat_in = [np.concatenate([np.asarray(m[n]) for m in in_maps],
                                    axis=0) for n in in_names]
        concat_zeros = [np.zeros((n_cores * z.shape[0], *z.shape[1:]),
                                 z.dtype) for z in zero_outs]
        outs = fn(*[jax.device_put(a, sh) for a in concat_in],
                  *[jax.device_put(z, sh) for z in concat_zeros])
        return {name: np.asarray(outs[i])
                for i, name in enumerate(out_names)}

    return run


_RUNNER_CACHE = {}


def kernel(x, gamma, beta, Wq, bq, Wk, bk, Wv, bv, Wp, bp):
    x = np.asarray(x, np.float32)
    B, H, W, C = x.shape
    S = H * W
    assert B % N_CORES == 0
    B_loc = B // N_CORES

    use_bv = bool(np.any(np.asarray(bv)))
    use_bp = bool(np.any(np.asarray(bp)))
    key = (B_loc, S, C, use_bv, use_bp)
    if key not in _RUNNER_CACHE:
        nc = _get_compiled_nc(B_loc, S, C, use_bv, use_bp)
        _RUNNER_CACHE[key] = _make_runner(nc, N_CORES)
    run = _RUNNER_CACHE[key]

    shared = make_const_inputs(C=C)
    shared.update(make_weight_inputs(Wq, bq, Wk, bk, Wv, bv, Wp, bp,
                                     gamma, beta))
    xr = x.reshape(B, S, C)
    in_maps = [
        {**shared, "x": np.ascontiguousarray(xr[k * B_loc:(k + 1) * B_loc])}
        for k in range(N_CORES)
    ]
    out = run(in_maps)
    y = out["y"].reshape(N_CORES, B_loc, S, C).reshape(B, S, C)
    return np.ascontiguousarray(y.reshape(B, H, W, C).astype(np.float32))
